# revision 6
# baseline (speedup 1.0000x reference)
"""Masked attention (B=16, QT=KT=2048, D=1024, fp32) on 8 Trainium2 NeuronCores.

Strategy:
 - Work unit = (128 q rows) x (512 k cols) partial attention with online
   (flash-style) softmax accumulation; k-outer / q-inner inside a "fragment".
 - A fragment = (NQ q-tiles) x (NK k-chunks) of one batch; every core runs an
   identical static sequence of fragment shapes (SPMD), host packs which
   (batch, q-range) goes where, padding with dummy slots.
 - Per-batch length specialization: only k-chunks < ceil(K_len/512) and
   q-tiles < ceil(Q_len/128) are computed. Invalid q rows are fixed up on the
   host (reference semantics: fully-masked rows -> uniform average of V).
 - Numerics: S = Q.K^T via 3-pass bf16 hi/lo split (logit err ~1e-3);
   exp/softmax in fp32 on ScalarE; A.V in tf32 (float32r). End-to-end absmax
   relative error vs the fp32 reference ~5e-4.
"""

import os
import numpy as np
import ml_dtypes
from contextlib import ExitStack

import concourse.bass as bass
import concourse.tile as tile
from concourse import bacc, mybir
from concourse.bass_utils import run_bass_kernel_spmd
from concourse.masks import make_identity

F32 = mybir.dt.float32
F32R = mybir.dt.float32r
BF16 = mybir.dt.bfloat16
AF = mybir.ActivationFunctionType
ALU = mybir.AluOpType

B, QT, KT, D = 16, 2048, 2048, 1024
QTILE, KCH = 128, 512
NCORES = 8
DCH = D // 128          # 8 contraction chunks of 128
KBLK = KCH // 128       # 4 k sub-blocks per chunk (transpose granularity)
MASK_NEG = float(-(2 ** 32))
NQ_MAX = 6

_PROG_CACHE: dict = {}
LAST_EXEC_NS = [None]


def _tf32(x):
    b = np.ascontiguousarray(x, dtype=np.float32).view(np.uint32)
    rb = (b >> 13) & np.uint32(1)
    b = (b + np.uint32(0x0FFF) + rb) & np.uint32(0xFFFFE000)
    return b.view(np.float32)


# --------------------------------------------------------------------------
# planning: choose fragment shape classes + assign (batch, q-run) fragments
# --------------------------------------------------------------------------

def _plan(nqt, nkt):
    """Returns (groups, assign):
    groups: list of (NK, NQ, F) executed by every core in order.
    assign: dict (core, group_idx, frag_idx) -> (batch, q_tile_start, run_len)
    """
    batches = list(range(len(nqt)))
    distinct = sorted({nkt[b] for b in batches})

    def group_cost(nks):
        """best (cost, NQ, F) for the batch set with nkt in nks."""
        bs = [b for b in batches if nkt[b] in nks]
        if not bs:
            return (0, 0, 0)
        NK = max(nkt[b] for b in bs)
        best = None
        for NQ in range(1, NQ_MAX + 1):
            nfr = sum(-(-nqt[b] // NQ) for b in bs)
            F = -(-nfr // NCORES)
            cost = F * NQ * NK
            if best is None or cost < best[0] or (cost == best[0] and F < best[2]):
                best = (cost, NQ, F)
        return best

    # try all contiguous partitions of the distinct nkt values into groups
    best_total, best_parts = None, None
    n = len(distinct)
    for mask in range(1 << max(0, n - 1)):
        parts, start = [], 0
        for i in range(n - 1):
            if mask >> i & 1:
                parts.append(distinct[start:i + 1])
                start = i + 1
        parts.append(distinct[start:])
        total = sum(group_cost(p)[0] for p in parts)
        if best_total is None or total < best_total:
            best_total, best_parts = total, parts

    groups, assign = [], {}
    gi = 0
    for part in best_parts:
        bs = [b for b in batches if nkt[b] in part]
        if not bs:
            continue
        NK = max(nkt[b] for b in bs)
        _, NQ, F = group_cost(part)
        groups.append((NK, NQ, F))
        # runs: split each batch's q tiles into runs of <= NQ
        runs = []
        for b in sorted(bs, key=lambda b: -nqt[b]):
            q = 0
            while q < nqt[b]:
                ln = min(NQ, nqt[b] - q)
                runs.append((b, q, ln))
                q += ln
        assert len(runs) <= NCORES * F
        for i, run in enumerate(runs):
            assign[(i % NCORES, gi, i // NCORES)] = run
        gi += 1
    return groups, assign


# --------------------------------------------------------------------------
# device program (cached by fragment-shape signature)
# --------------------------------------------------------------------------

def _build_program(groups):
    TQ = sum(NQ * F for (_, NQ, F) in groups)
    CH = sum(NK * F for (NK, _, F) in groups)

    nc = bacc.Bacc("TRN2", target_bir_lowering=False, debug=False)
    qh_e = nc.dram_tensor("qh", [TQ, DCH, 128, QTILE], BF16, kind="ExternalInput")
    ql_e = nc.dram_tensor("ql", [TQ, DCH, 128, QTILE], BF16, kind="ExternalInput")
    kh_e = nc.dram_tensor("kh", [CH, DCH, 128, KCH], BF16, kind="ExternalInput")
    kl_e = nc.dram_tensor("kl", [CH, DCH, 128, KCH], BF16, kind="ExternalInput")
    v_e = nc.dram_tensor("v", [CH, KBLK, 128, D], F32R, kind="ExternalInput")
    mk_e = nc.dram_tensor("mask", [CH, 128, KCH], BF16, kind="ExternalInput")
    o_e = nc.dram_tensor("o", [TQ, 128, D], F32, kind="ExternalOutput")

    with tile.TileContext(nc) as tc:
        with ExitStack() as ctx:
            const = ctx.enter_context(tc.tile_pool(name="const", bufs=1))
            qpool = ctx.enter_context(tc.tile_pool(name="qpool", bufs=2))
            kpool = ctx.enter_context(tc.tile_pool(name="kpool", bufs=2))
            vpool = ctx.enter_context(tc.tile_pool(name="vpool", bufs=2))
            mpool = ctx.enter_context(tc.tile_pool(name="mpool", bufs=3))
            state = ctx.enter_context(tc.tile_pool(name="state", bufs=2))
            work = ctx.enter_context(tc.tile_pool(name="work", bufs=3))
            small = ctx.enter_context(tc.tile_pool(name="small", bufs=6))
            opool = ctx.enter_context(tc.tile_pool(name="opool", bufs=2))
            ps_s = ctx.enter_context(tc.tile_pool(name="ps_s", bufs=2, space="PSUM"))
            ps_t = ctx.enter_context(tc.tile_pool(name="ps_t", bufs=2, space="PSUM"))
            ps_o = ctx.enter_context(tc.tile_pool(name="ps_o", bufs=2, space="PSUM"))

            ident = const.tile([128, 128], F32)
            make_identity(nc, ident)

            qslot = 0
            chslot = 0
            for (NK, NQ, F) in groups:
                for f in range(F):
                    # fragment state
                    mbar = state.tile([128, NQ], F32, tag="mbar")
                    dst = state.tile([128, NQ], F32, tag="dst")
                    oacc = state.tile([128, NQ * D], F32, tag="oacc")

                    # load this fragment's q tiles
                    qh = qpool.tile([128, NQ, DCH, QTILE], BF16, tag="qh")
                    ql = qpool.tile([128, NQ, DCH, QTILE], BF16, tag="ql")
                    for t in range(NQ):
                        nc.sync.dma_start(
                            qh[:, t], qh_e[qslot + t].rearrange("c p q -> p c q"))
                        nc.sync.dma_start(
                            ql[:, t], ql_e[qslot + t].rearrange("c p q -> p c q"))

                    for j in range(NK):
                        kh = kpool.tile([128, DCH, KCH], BF16, tag="kh")
                        kl = kpool.tile([128, DCH, KCH], BF16, tag="kl")
                        vv = vpool.tile([128, KBLK, D], F32R, tag="v")
                        mk = mpool.tile([128, KCH], BF16, tag="mk")
                        nc.sync.dma_start(kh[:], kh_e[chslot + j].rearrange("c p k -> p c k"))
                        nc.sync.dma_start(kl[:], kl_e[chslot + j].rearrange("c p k -> p c k"))
                        nc.sync.dma_start(vv[:], v_e[chslot + j].rearrange("c p d -> p c d"))
                        nc.sync.dma_start(mk[:], mk_e[chslot + j])

                        for t in range(NQ):
                            # S = Qh.Kh + Qh.Kl + Ql.Kh  (24 matmuls, fp32 PSUM)
                            sp = ps_s.tile([128, KCH], F32, tag="sp")
                            i = 0
                            for c in range(DCH):
                                for (lhs, rhs) in ((qh, kh), (qh, kl), (ql, kh)):
                                    nc.tensor.matmul(
                                        sp[:], lhs[:, t, c], rhs[:, c],
                                        start=(i == 0), stop=(i == 3 * DCH - 1))
                                    i += 1

                            s_sb = work.tile([128, KCH], F32, tag="s_sb")
                            nc.vector.tensor_add(s_sb[:], sp[:], mk[:])
                            mbj = small.tile([128, 1], F32, tag="mbj")
                            nc.vector.tensor_reduce(
                                mbj[:], s_sb[:], axis=mybir.AxisListType.X,
                                op=ALU.max, negate=True)

                            st = slice(t, t + 1)
                            if j == 0:
                                nc.vector.tensor_copy(mbar[:, st], mbj[:])
                                mnew = mbj
                            else:
                                mnew = small.tile([128, 1], F32, tag="mnew")
                                nc.vector.tensor_tensor(
                                    mnew[:], mbj[:], mbar[:, st], ALU.min)
                                alpha = small.tile([128, 1], F32, tag="alpha")
                                # alpha = exp(m_old - m_new) = exp(mnew_bar - mold_bar)
                                nc.scalar.activation(
                                    alpha[:], mbar[:, st], AF.Exp,
                                    bias=mnew[:], scale=-1.0)
                                nc.vector.tensor_copy(mbar[:, st], mnew[:])

                            # P = exp(S - m), row sums
                            p_sb = work.tile([128, KCH], F32, tag="p_sb")
                            sj = small.tile([128, 1], F32, tag="sj")
                            nc.scalar.activation(
                                p_sb[:], s_sb[:], AF.Exp, bias=mnew[:], scale=1.0,
                                accum_out=sj[:])

                            if j == 0:
                                nc.vector.tensor_copy(dst[:, st], sj[:])
                            else:
                                nc.vector.scalar_tensor_tensor(
                                    out=dst[:, st], in0=dst[:, st], scalar=alpha[:],
                                    in1=sj[:], op0=ALU.mult, op1=ALU.add)

                            # transpose P blocks -> f32r
                            pt = work.tile([128, KBLK, 128], F32R, tag="pt")
                            for kb in range(KBLK):
                                ptp = ps_t.tile([128, 128], F32, tag="ptp")
                                nc.tensor.transpose(
                                    ptp[:], p_sb[:, bass.ts(kb, 128)], ident[:])
                                nc.vector.tensor_copy(pt[:, kb], ptp[:])

                            # O_j = P^T-blocks @ V
                            op = ps_o.tile([128, D], F32, tag="op")
                            for dh in range(2):
                                for kb in range(KBLK):
                                    nc.tensor.matmul(
                                        op[:, bass.ds(dh * 512, 512)],
                                        pt[:, kb], vv[:, kb, bass.ds(dh * 512, 512)],
                                        start=(kb == 0), stop=(kb == KBLK - 1))

                            ot = slice(t * D, (t + 1) * D)
                            if j == 0:
                                nc.vector.tensor_copy(oacc[:, ot], op[:])
                            else:
                                nc.vector.scalar_tensor_tensor(
                                    out=oacc[:, ot], in0=oacc[:, ot], scalar=alpha[:],
                                    in1=op[:], op0=ALU.mult, op1=ALU.add)

                    # finalize fragment: O_final = oacc / d
                    for t in range(NQ):
                        rec = small.tile([128, 1], F32, tag="rec")
                        nc.vector.reciprocal(rec[:], dst[:, t:t + 1])
                        ofin = opool.tile([128, D], F32, tag="ofin")
                        nc.scalar.activation(
                            ofin[:], oacc[:, t * D:(t + 1) * D], AF.Copy,
                            bias=0.0, scale=rec[:])
                        nc.sync.dma_start(o_e[qslot + t], ofin[:])

                    qslot += NQ
                    chslot += NK

    nc.compile()
    return nc, TQ, CH


# --------------------------------------------------------------------------
# cached PJRT executor (adapted from concourse.bass2jax.run_bass_via_pjrt)
# --------------------------------------------------------------------------

_EXEC_CACHE: dict = {}


def _get_exec(nc):
    import jax
    from concourse import bass2jax, mybir as _mb
    from jax.experimental.shard_map import shard_map
    from jax.sharding import Mesh, PartitionSpec

    key = id(nc)
    if key in _EXEC_CACHE:
        return _EXEC_CACHE[key]
    bass2jax.install_neuronx_cc_hook()
    assert not nc.dbg_addr or not nc.dbg_callbacks

    partition_name = nc.partition_id_tensor.name if nc.partition_id_tensor else None
    in_names, out_names, out_avals = [], [], []
    for alloc in nc.m.functions[0].allocations:
        if not isinstance(alloc, _mb.MemoryLocationSet):
            continue
        name = alloc.memorylocations[0].name
        if alloc.kind == "ExternalInput":
            if name != partition_name:
                in_names.append(name)
        elif alloc.kind == "ExternalOutput":
            shape = tuple(alloc.tensor_shape)
            dtype = _mb.dt.np(alloc.dtype)
            out_names.append(name)
            out_avals.append(jax.core.ShapedArray(shape, dtype))
    n_params = len(in_names)
    n_outs = len(out_avals)
    all_in_names = list(in_names) + list(out_names)
    if partition_name is not None:
        all_in_names.append(partition_name)
    donate = tuple(range(n_params, n_params + n_outs))

    def _body(*args):
        operands = list(args)
        if partition_name is not None:
            operands.append(bass2jax.partition_id_tensor())
        return tuple(bass2jax._bass_exec_p.bind(
            *operands,
            out_avals=tuple(out_avals),
            in_names=tuple(all_in_names),
            out_names=tuple(out_names),
            lowering_input_output_aliases=(),
            sim_require_finite=True,
            sim_require_nnan=True,
            nc=nc,
        ))

    devices = jax.devices()[:NCORES]
    mesh = Mesh(np.asarray(devices), ("core",))
    in_specs = (PartitionSpec("core"),) * (n_params + n_outs)
    out_specs = (PartitionSpec("core"),) * n_outs
    sharded = jax.jit(
        shard_map(_body, mesh=mesh, in_specs=in_specs, out_specs=out_specs,
                  check_rep=False),
        donate_argnums=donate, keep_unused=True)
    info = dict(sharded=sharded, in_names=in_names, out_names=out_names,
                out_avals=out_avals, mesh=mesh, n_params=n_params)
    _EXEC_CACHE[key] = info
    return info


def _concat_inputs(info, in_maps):
    return [np.concatenate([np.asarray(m[name]) for m in in_maps], axis=0)
            for name in info["in_names"]]


def _zero_outs(info):
    return [np.zeros((NCORES * a.shape[0], *a.shape[1:]), a.dtype)
            for a in info["out_avals"]]


def _execute(nc, in_maps):
    import jax
    info = _get_exec(nc)
    concat_in = _concat_inputs(info, in_maps)
    out_arrs = info["sharded"](*concat_in, *_zero_outs(info))
    results = [
        {name: np.asarray(out_arrs[i]).reshape(NCORES, *info["out_avals"][i].shape)[c]
         for i, name in enumerate(info["out_names"])}
        for c in range(NCORES)
    ]
    if int(os.environ.get("ATTN_TIME", "0")):
        LAST_EXEC_NS[0] = _time_exec(nc, concat_in, int(os.environ.get("ATTN_TIME_ITERS", "3")))
    return results


def _time_exec(nc, concat_in, iters=3):
    """Wall-clock the sharded execution with device-resident inputs."""
    import time
    import jax
    from jax.sharding import NamedSharding, PartitionSpec
    info = _get_exec(nc)
    sh = NamedSharding(info["mesh"], PartitionSpec("core"))
    dev_in = [jax.device_put(x, sh) for x in concat_in]
    for x in dev_in:
        x.block_until_ready()
    times = []
    for _ in range(iters):
        zeros = [jax.device_put(z, sh) for z in _zero_outs(info)]
        for z in zeros:
            z.block_until_ready()
        t0 = time.perf_counter()
        outs = info["sharded"](*dev_in, *zeros)
        for o in outs:
            o.block_until_ready()
        times.append(time.perf_counter() - t0)
    best = min(times)
    print(f"exec wall times: {[f'{t*1e3:.2f}ms' for t in times]}")
    return int(best * 1e9)


# --------------------------------------------------------------------------
# host entry
# --------------------------------------------------------------------------

def kernel(Q, K, V, Q_lengths, K_lengths):
    Q = np.ascontiguousarray(np.asarray(Q, dtype=np.float32))
    K = np.ascontiguousarray(np.asarray(K, dtype=np.float32))
    V = np.ascontiguousarray(np.asarray(V, dtype=np.float32))
    ql_i = np.asarray(Q_lengths).astype(np.int64)
    kl_i = np.asarray(K_lengths).astype(np.int64)

    nqt = [int(-(-min(max(q, 0), QT) // QTILE)) for q in ql_i]
    nkt = [int(-(-min(max(k, 1), KT) // KCH)) for k in kl_i]

    groups, assign = _plan(nqt, nkt)
    sig = tuple(groups)
    if sig not in _PROG_CACHE:
        _PROG_CACHE[sig] = _build_program(groups)
    nc, TQ, CH = _PROG_CACHE[sig]

    # precompute split/rounded operands
    Qh = Q.astype(ml_dtypes.bfloat16)
    Ql = (Q - Qh.astype(np.float32)).astype(ml_dtypes.bfloat16)
    Kh = K.astype(ml_dtypes.bfloat16)
    Kl = (K - Kh.astype(np.float32)).astype(ml_dtypes.bfloat16)
    Vr = _tf32(V)

    in_maps = []
    for c in range(NCORES):
        qh_a = np.zeros((TQ, DCH, 128, QTILE), dtype=ml_dtypes.bfloat16)
        ql_a = np.zeros_like(qh_a)
        kh_a = np.zeros((CH, DCH, 128, KCH), dtype=ml_dtypes.bfloat16)
        kl_a = np.zeros_like(kh_a)
        v_a = np.zeros((CH, KBLK, 128, D), dtype=np.float32)
        mk_a = np.full((CH, 128, KCH), MASK_NEG, dtype=ml_dtypes.bfloat16)
        qslot = chslot = 0
        for gi, (NK, NQ, F) in enumerate(groups):
            for f in range(F):
                run = assign.get((c, gi, f))
                if run is not None:
                    b, q0, ln = run
                    for t in range(ln):
                        qt = q0 + t
                        blk = Qh[b, qt * QTILE:(qt + 1) * QTILE, :].T  # [D, 128]
                        qh_a[qslot + t] = blk.reshape(DCH, 128, QTILE)
                        blk = Ql[b, qt * QTILE:(qt + 1) * QTILE, :].T
                        ql_a[qslot + t] = blk.reshape(DCH, 128, QTILE)
                    klen = int(min(max(kl_i[b], 1), KT))
                    for j in range(nkt[b]):
                        ksl = slice(j * KCH, (j + 1) * KCH)
                        kh_a[chslot + j] = Kh[b, ksl, :].T.reshape(DCH, 128, KCH)
                        kl_a[chslot + j] = Kl[b, ksl, :].T.reshape(DCH, 128, KCH)
                        v_a[chslot + j] = Vr[b, ksl, :].reshape(KBLK, 128, D)
                        kk = np.arange(j * KCH, (j + 1) * KCH)
                        row = np.where(kk < klen, 0.0, MASK_NEG).astype(
                            ml_dtypes.bfloat16)
                        mk_a[chslot + j] = np.broadcast_to(row, (128, KCH))
                qslot += NQ
                chslot += NK
        in_maps.append({"qh": qh_a, "ql": ql_a, "kh": kh_a, "kl": kl_a,
                        "v": v_a, "mask": mk_a})

    results = _execute(nc, in_maps)

    # assemble output
    out = np.empty((B, QT, D), dtype=np.float32)
    v_mean = V.mean(axis=1, dtype=np.float64).astype(np.float32)  # [B, D]
    done = np.zeros((B, QT // QTILE), dtype=bool)
    for c in range(NCORES):
        o_a = results[c]["o"]
        qslot = 0
        for gi, (NK, NQ, F) in enumerate(groups):
            for f in range(F):
                run = assign.get((c, gi, f))
                if run is not None:
                    b, q0, ln = run
                    for t in range(ln):
                        out[b, (q0 + t) * QTILE:(q0 + t + 1) * QTILE, :] = \
                            o_a[qslot + t]
                        done[b, q0 + t] = True
                qslot += NQ
    # rows q >= Q_len: reference yields uniform average over ALL of V
    for b in range(B):
        qlen = int(min(max(ql_i[b], 0), QT))
        out[b, qlen:, :] = v_mean[b]
        assert done[b, :nqt[b]].all()
    return out
